# revision 6
# baseline (speedup 1.0000x reference)
"""Cross-attention reducer kernel for Trainium2, 8 NeuronCores (SPMD).

Problem (full shapes):
    token_input    [T=8192, L=4096]
    learned_queries[V=4096, I=512]
    w_q [I, I], w_k [L, I], w_v [L, I], w_out [I, L]

    q = learned_queries @ w_q;  k = token_input @ w_k;  v = token_input @ w_v
    per head h (H=8, D=64): attn = softmax(q_h k_h^T / sqrt(D)); out_h = attn @ v_h
    out = concat_h(out_h) @ w_out      -> [V, L]

Sharding: queries (V) are sharded 8 ways; the K/V projections are
sequence-parallel (each core projects its T/8 token shard with the full
w_k/w_v) followed by an AllGather of k^T and v, after which every core runs
attention for all 8 heads over its own 512 queries and its full-T gathered
k/v, then applies the output projection for its V-shard. The final output is
computed transposed (final^T = w_out^T-contraction) so every matmul contracts
on the partition dimension with no large transposes anywhere:

    q^T  [I, Vs]  = w_q (lhsT)  x lq^T (rhs)
    k^T  [I, t]   = w_k (lhsT)  x tok^T (rhs)        (per T-shard, gathered)
    v^T  [I, t]   = w_v (lhsT)  x tok^T (rhs), then 128x128 PE-transposes
                    to v [t, I] before the gather
    s^T  [t, Vs]  = k_h^T (lhsT) x q_h^T (rhs)       (t-tiles of 128)
    p^T           = exp(s^T / 8)                      (no max-subtraction:
                    scores are O(3), exp can't overflow; identical math)
    u^T  [D+1,Vs] = [v_h | 1] (lhsT) x p^T (rhs)     (row D = softmax denom)
    a^T  [D, Vs]  = u^T * (1/denom broadcast via PE outer product)
    out^T[L, Vs]  = w_out (lhsT) x a^T (rhs)

Matmul dtypes: fp32r (full-rate fp32, ~6.5e-5 rel err) for all projections
and the output projection; bf16 for the gathered k/v + attention matmuls
(halves gather traffic; ~0.5% worst-case contribution, well under tolerance).
"""

import os

import numpy as np

import concourse.bacc as bacc
import concourse.tile as tile
import concourse.mybir as mybir
from concourse.bass_utils import run_bass_kernel_spmd

F32 = mybir.dt.float32
F32R = mybir.dt.float32r
BF16 = mybir.dt.bfloat16
EXP = mybir.ActivationFunctionType.Exp
EQ = mybir.AluOpType.is_equal

N_CORES = 8
T, L, V, INNER = 8192, 4096, 4096, 512
H, D = 8, 64
TS = T // N_CORES      # 1024  t-shard per core
QS = V // N_CORES      # 512   query shard per core
SCALE = D ** -0.5      # 0.125

NT = T // 128          # 64 t-tiles per head
GRP = 3                # t-tiles per exp batch (3 psum banks)

# diagnostics: BASSK_F32=1 -> plain f32 matmuls + sync-engine loads (no casts)
_USE_F32 = bool(os.environ.get("BASSK_F32"))


def build_program():
    FR = F32 if _USE_F32 else F32R
    ld = None  # set after nc exists
    nc = bacc.Bacc(
        "TRN2", target_bir_lowering=False, debug=False, num_devices=N_CORES
    )

    tok_T = nc.dram_tensor("tok_T", [L, TS], F32, kind="ExternalInput").ap()
    lq_T = nc.dram_tensor("lq_T", [INNER, QS], F32, kind="ExternalInput").ap()
    w_q = nc.dram_tensor("w_q", [INNER, INNER], F32, kind="ExternalInput").ap()
    w_k = nc.dram_tensor("w_k", [L, INNER], F32, kind="ExternalInput").ap()
    w_v = nc.dram_tensor("w_v", [L, INNER], F32, kind="ExternalInput").ap()
    w_out = nc.dram_tensor("w_out", [INNER, L], F32, kind="ExternalInput").ap()
    outT = nc.dram_tensor("outT", [L, QS], F32, kind="ExternalOutput").ap()

    # rearranged DRAM views (partition-major for SBUF loads)
    tok_T_v = tok_T.rearrange("(k p) t -> p k t", p=128)        # [128, 32, 1024]
    lq_v = lq_T.rearrange("(k p) q -> p k q", p=128)            # [128, 4, 512]
    w_q_v = w_q.rearrange("(k p) i -> p k i", p=128)            # [128, 4, 512]
    w_k_v = w_k.rearrange("(k p) i -> p k i", p=128)            # [128, 32, 512]
    w_v_v = w_v.rearrange("(k p) i -> p k i", p=128)            # [128, 32, 512]
    w_out_v = w_out.rearrange("(k p) l -> p k l", p=128)        # [128, 4, 4096]

    ld_eng = (lambda: nc.sync) if _USE_F32 else (lambda: nc.gpsimd)

    with tile.TileContext(nc) as tc:
        with (
            tc.tile_pool(name="persist", bufs=1) as persist,
            tc.tile_pool(name="dram", bufs=1, space="DRAM") as dram,
        ):
            # ---- persistent SBUF across phases ----
            qT_sb = persist.tile([64, H, QS], BF16, tag="qT")        # q^T per head
            kT_sb = persist.tile([128, 4, TS], BF16, tag="kT")       # k^T shard
            v_sb = persist.tile([128, TS // 128, INNER], BF16, tag="v")  # v shard
            aT_sb = persist.tile([128, 4, QS], FR, tag="aT")       # attn out^T
            idn = persist.tile([128, 128], BF16, tag="idn")          # identity
            ones_64 = persist.tile([1, D], F32, tag="ones64")

            # collective bounce buffers
            gk_in = dram.tile([INNER, TS], BF16, tag="gk_in")
            gk_out = dram.tile([N_CORES * INNER, TS], BF16, tag="gk_out")
            gv_in = dram.tile([TS, INNER], BF16, tag="gv_in")
            gv_out = dram.tile([N_CORES * TS, INNER], BF16, tag="gv_out")

            # identity matrix for PE transposes: idn[p, f] = (f == p)
            with tc.tile_pool(name="idpool", bufs=1) as idp:
                irow = idp.tile([128, 128], F32, tag="irow")
                icol = idp.tile([128, 1], F32, tag="icol")
                nc.gpsimd.iota(irow[:], pattern=[[1, 128]], base=0, channel_multiplier=0, allow_small_or_imprecise_dtypes=True)
                nc.gpsimd.iota(icol[:], pattern=[[0, 1]], base=0, channel_multiplier=1, allow_small_or_imprecise_dtypes=True)
                nc.vector.tensor_scalar(idn[:], irow[:], icol[:], None, EQ)
            nc.vector.memset(ones_64[:], 1.0)

            # ================= phase 1: projections =================
            with (
                tc.tile_pool(name="proj", bufs=2) as proj,
                tc.tile_pool(name="projq", bufs=1) as projq,
                tc.tile_pool(name="pps", bufs=2, space="PSUM") as pps,
                tc.tile_pool(name="ppq", bufs=1, space="PSUM") as ppq,
            ):
                # --- q^T = w_q^T-contraction: lhsT=w_q tile, rhs=lq^T tile
                wq_sb = projq.tile([128, 4, INNER], FR, tag="wq")
                lq_sb = projq.tile([128, 4, QS], FR, tag="lq")
                ld_eng().dma_start(wq_sb[:], w_q_v)
                ld_eng().dma_start(lq_sb[:], lq_v)
                for m in range(4):
                    ps = ppq.tile([128, QS], F32, tag="psq")
                    for kk in range(4):
                        nc.tensor.matmul(
                            ps[:],
                            wq_sb[:, kk, m * 128:(m + 1) * 128],
                            lq_sb[:, kk, :],
                            start=(kk == 0),
                            stop=(kk == 3),
                        )
                    qstage = projq.tile([128, QS], BF16, tag="qstage")
                    nc.vector.tensor_copy(qstage[:], ps[:])
                    # shift each head's 64 rows down to base partition 0
                    nc.sync.dma_start(qT_sb[:, 2 * m, :], qstage[0:64, :])
                    nc.sync.dma_start(qT_sb[:, 2 * m + 1, :], qstage[64:128, :])

                # --- k^T and v^T projections, t streamed in two halves
                for th in range(2):
                    tok_sb = proj.tile([128, 32, 512], FR, tag="tok", bufs=1)
                    ld_eng().dma_start(tok_sb[:], tok_T_v[:, :, th * 512:(th + 1) * 512])
                    # k^T [i-block m, t-half]
                    for m in range(4):
                        wcol = proj.tile([128, 32, 128], FR, tag="wcol")
                        ld_eng().dma_start(wcol[:], w_k_v[:, :, m * 128:(m + 1) * 128])
                        ps = pps.tile([128, 512], F32, tag="pp")
                        for k in range(32):
                            nc.tensor.matmul(
                                ps[:], wcol[:, k, :], tok_sb[:, k, :],
                                start=(k == 0), stop=(k == 31),
                            )
                        nc.vector.tensor_copy(kT_sb[:, m, th * 512:(th + 1) * 512], ps[:])
                    # v^T then transpose to v [t, i]
                    for m in range(4):
                        wcol = proj.tile([128, 32, 128], FR, tag="wcol")
                        ld_eng().dma_start(wcol[:], w_v_v[:, :, m * 128:(m + 1) * 128])
                        ps = pps.tile([128, 512], F32, tag="pp")
                        for k in range(32):
                            nc.tensor.matmul(
                                ps[:], wcol[:, k, :], tok_sb[:, k, :],
                                start=(k == 0), stop=(k == 31),
                            )
                        vst = proj.tile([128, 512], BF16, tag="vstage")
                        nc.vector.tensor_copy(vst[:], ps[:])
                        pt = pps.tile([128, 512], BF16, tag="pt")
                        for j in range(4):
                            nc.tensor.transpose(
                                pt[:, j * 128:(j + 1) * 128],
                                vst[:, j * 128:(j + 1) * 128],
                                idn[:],
                            )
                        # pt columns j hold v[t-chunk j of this half, i-block m]
                        nc.vector.tensor_copy(
                            v_sb[:, th * 4:(th + 1) * 4, m * 128:(m + 1) * 128],
                            pt[:].rearrange("p (j i) -> p j i", j=4),
                        )

                # bounce + gather
                nc.sync.dma_start(
                    gk_in.rearrange("(m p) t -> p m t", p=128), kT_sb[:]
                )
                nc.sync.dma_start(
                    gv_in.rearrange("(j p) i -> p j i", p=128), v_sb[:]
                )
                if os.environ.get("BASSK_NO_CC"):
                    # timing-only variant: skip the collectives (wrong data)
                    nc.sync.dma_start(gk_out[0:INNER, :], gk_in[:])
                    nc.sync.dma_start(gv_out[0:TS, :], gv_in[:])
                else:
                    nc.gpsimd.collective_compute(
                        "AllGather", mybir.AluOpType.bypass,
                        replica_groups=[list(range(N_CORES))],
                        ins=[gk_in.opt()], outs=[gk_out.opt()],
                    )
                    nc.gpsimd.collective_compute(
                        "AllGather", mybir.AluOpType.bypass,
                        replica_groups=[list(range(N_CORES))],
                        ins=[gv_in.opt()], outs=[gv_out.opt()],
                    )

            # ================= phase 2: attention =================
            gv_v = gv_out.rearrange("(c j p) i -> c p j i", p=128, j=TS // 128)
            groups = [list(range(s, min(s + GRP, NT))) for s in range(0, NT, GRP)]

            with (
                tc.tile_pool(name="attn", bufs=2) as attn,
                tc.tile_pool(name="attn3", bufs=3) as attn3,
                tc.tile_pool(name="aps", bufs=2, space="PSUM") as aps,
                tc.tile_pool(name="aps1", bufs=1, space="PSUM") as aps1,
            ):
                for h in range(H):
                    kTh = attn.tile([64, N_CORES, TS], BF16, tag="kTh")
                    for c in range(N_CORES):
                        nc.sync.dma_start(
                            kTh[:, c, :],
                            gk_out[c * INNER + h * D: c * INNER + h * D + D, :],
                        )
                    vh = attn.tile([128, NT, D + 1], BF16, tag="vh")
                    nc.vector.memset(vh[:, :, D], 1.0)
                    for c in range(N_CORES):
                        nc.sync.dma_start(
                            vh[:, c * (TS // 128):(c + 1) * (TS // 128), 0:D],
                            gv_v[c, :, :, h * D:(h + 1) * D],
                        )
                    qTh = qT_sb[:, h, :]

                    ps_o = aps1.tile([D + 1, QS], F32, tag="ps_o")
                    prev = None  # (group, pT tile)
                    for g in groups:
                        ps_s = aps.tile([128, GRP * QS], F32, tag="ps_s")
                        for jj, j in enumerate(g):
                            nc.tensor.matmul(
                                ps_s[:, jj * QS:(jj + 1) * QS],
                                kTh[:, j // (TS // 128), (j % (TS // 128)) * 128:
                                    (j % (TS // 128)) * 128 + 128],
                                qTh,
                                start=True, stop=True,
                            )
                        pT = attn3.tile([128, GRP * QS], BF16, tag="pT")
                        n = len(g) * QS
                        nc.scalar.activation(pT[:, 0:n], ps_s[:, 0:n], EXP, scale=SCALE)
                        if prev is not None:
                            pg, ppT = prev
                            for jj, j in enumerate(pg):
                                nc.tensor.matmul(
                                    ps_o[:], vh[:, j, :], ppT[:, jj * QS:(jj + 1) * QS],
                                    start=(j == 0), stop=(j == NT - 1),
                                    skip_group_check=True,
                                )
                        prev = (g, pT)
                    pg, ppT = prev
                    for jj, j in enumerate(pg):
                        nc.tensor.matmul(
                            ps_o[:], vh[:, j, :], ppT[:, jj * QS:(jj + 1) * QS],
                            start=(j == 0), stop=(j == NT - 1),
                            skip_group_check=True,
                        )

                    # normalize: a^T = u^T / denom  (denom broadcast via PE)
                    u_sb = attn.tile([D + 1, QS], F32, tag="u")
                    nc.vector.tensor_copy(u_sb[:], ps_o[:])
                    dn0 = attn.tile([1, QS], F32, tag="dn0")
                    nc.sync.dma_start(dn0[:], u_sb[D:D + 1, :])  # shift to partition 0
                    recip = attn.tile([1, QS], F32, tag="recip")
                    nc.vector.reciprocal(recip[:], dn0[:])
                    ps_r = aps.tile([D, QS], F32, tag="ps_s")  # borrow a ps_s slot
                    nc.tensor.matmul(ps_r[:], ones_64[:], recip[:], start=True, stop=True)
                    a_tmp = attn.tile([D, QS], F32, tag="a_tmp")
                    nc.vector.tensor_mul(a_tmp[:], u_sb[0:D, :], ps_r[:])
                    ld_eng().dma_start(
                        aT_sb[(h % 2) * 64:(h % 2) * 64 + 64, h // 2, :], a_tmp[:]
                    )

            # ================= phase 3: output projection =================
            with (
                tc.tile_pool(name="outp", bufs=3) as outp,
                tc.tile_pool(name="ops", bufs=2, space="PSUM") as ops,
            ):
                for m in range(L // 128):
                    wo = outp.tile([128, 4, 128], FR, tag="wo")
                    ld_eng().dma_start(wo[:], w_out_v[:, :, m * 128:(m + 1) * 128])
                    ps = ops.tile([128, QS], F32, tag="po")
                    for kk in range(4):
                        nc.tensor.matmul(
                            ps[:], wo[:, kk, :], aT_sb[:, kk, :],
                            start=(kk == 0), stop=(kk == 3),
                        )
                    of = outp.tile([128, QS], F32, tag="of")
                    nc.vector.tensor_copy(of[:], ps[:])
                    nc.sync.dma_start(outT[m * 128:(m + 1) * 128, :], of[:])

    nc.compile()
    return nc


_COMPILED = None


def _get_compiled():
    global _COMPILED
    if _COMPILED is None:
        _COMPILED = build_program()
    return _COMPILED


def make_in_maps(token_input, learned_queries, w_q, w_k, w_v, w_out):
    token_input = np.ascontiguousarray(np.asarray(token_input, dtype=np.float32))
    learned_queries = np.ascontiguousarray(np.asarray(learned_queries, dtype=np.float32))
    w_q = np.ascontiguousarray(np.asarray(w_q, dtype=np.float32))
    w_k = np.ascontiguousarray(np.asarray(w_k, dtype=np.float32))
    w_v = np.ascontiguousarray(np.asarray(w_v, dtype=np.float32))
    w_out = np.ascontiguousarray(np.asarray(w_out, dtype=np.float32))
    in_maps = []
    for c in range(N_CORES):
        in_maps.append({
            "tok_T": np.ascontiguousarray(token_input[c * TS:(c + 1) * TS, :].T),
            "lq_T": np.ascontiguousarray(learned_queries[c * QS:(c + 1) * QS, :].T),
            "w_q": w_q, "w_k": w_k, "w_v": w_v, "w_out": w_out,
        })
    return in_maps


def assemble(results):
    out = np.empty((V, L), dtype=np.float32)
    for c in range(N_CORES):
        out[c * QS:(c + 1) * QS, :] = results[c]["outT"].T
    return out


def kernel(token_input, learned_queries, w_q, w_k, w_v, w_out):
    nc = _get_compiled()
    in_maps = make_in_maps(token_input, learned_queries, w_q, w_k, w_v, w_out)
    res = run_bass_kernel_spmd(nc, in_maps, list(range(N_CORES)))
    return assemble(res.results)


# revision 7
# speedup vs baseline: 26.8052x; 26.8052x over previous
"""Cross-attention reducer kernel for Trainium2, 8 NeuronCores (SPMD).

Problem (full shapes):
    token_input    [T=8192, L=4096]
    learned_queries[V=4096, I=512]
    w_q [I, I], w_k [L, I], w_v [L, I], w_out [I, L]

    q = learned_queries @ w_q;  k = token_input @ w_k;  v = token_input @ w_v
    per head h (H=8, D=64): attn = softmax(q_h k_h^T / sqrt(D)); out_h = attn @ v_h
    out = concat_h(out_h) @ w_out      -> [V, L]

Sharding: queries (V) are sharded 8 ways; the K/V projections are
sequence-parallel (each core projects its T/8 token shard with the full
w_k/w_v) followed by an AllGather of k^T and v, after which every core runs
attention for all 8 heads over its own 512 queries and its full-T gathered
k/v, then applies the output projection for its V-shard. The final output is
computed transposed (final^T = w_out^T-contraction) so every matmul contracts
on the partition dimension with no large transposes anywhere:

    q^T  [I, Vs]  = w_q (lhsT)  x lq^T (rhs)
    k^T  [I, t]   = w_k (lhsT)  x tok^T (rhs)        (per T-shard, gathered)
    v^T  [I, t]   = w_v (lhsT)  x tok^T (rhs), then 128x128 PE-transposes
                    to v [t, I] before the gather
    s^T  [t, Vs]  = k_h^T (lhsT) x q_h^T (rhs)       (t-tiles of 128)
    p^T           = exp(s^T / 8)                      (no max-subtraction:
                    scores are O(3), exp can't overflow; identical math)
    u^T  [D+1,Vs] = [v_h | 1] (lhsT) x p^T (rhs)     (row D = softmax denom)
    a^T  [D, Vs]  = u^T * (1/denom broadcast via PE outer product)
    out^T[L, Vs]  = w_out (lhsT) x a^T (rhs)

Matmul dtypes: fp32r (full-rate fp32, ~6.5e-5 rel err) for all projections
and the output projection; bf16 for the gathered k/v + attention matmuls
(halves gather traffic; ~0.5% worst-case contribution, well under tolerance).
"""

import os

import numpy as np

import concourse.bacc as bacc
import concourse.tile as tile
import concourse.mybir as mybir
from concourse.bass_utils import run_bass_kernel_spmd

F32 = mybir.dt.float32
F32R = mybir.dt.float32r
BF16 = mybir.dt.bfloat16
EXP = mybir.ActivationFunctionType.Exp
EQ = mybir.AluOpType.is_equal

N_CORES = 8
T, L, V, INNER = 8192, 4096, 4096, 512
H, D = 8, 64
TS = T // N_CORES      # 1024  t-shard per core
QS = V // N_CORES      # 512   query shard per core
SCALE = D ** -0.5      # 0.125

NT = T // 128          # 64 t-tiles per head
GRP = 3                # t-tiles per exp batch (3 psum banks)

# diagnostics: BASSK_F32=1 -> plain f32 matmuls + sync-engine loads (no casts)
_USE_F32 = bool(os.environ.get("BASSK_F32"))
_PHASES = os.environ.get("BASSK_PHASES", "all")  # all | proj | attn


def build_program():
    FR = F32 if _USE_F32 else F32R
    ld = None  # set after nc exists
    nc = bacc.Bacc(
        "TRN2", target_bir_lowering=False, debug=False, num_devices=N_CORES
    )

    tok_T = nc.dram_tensor("tok_T", [L, TS], F32, kind="ExternalInput").ap()
    lq_T = nc.dram_tensor("lq_T", [INNER, QS], F32, kind="ExternalInput").ap()
    w_q = nc.dram_tensor("w_q", [INNER, INNER], F32, kind="ExternalInput").ap()
    w_k = nc.dram_tensor("w_k", [L, INNER], F32, kind="ExternalInput").ap()
    w_v = nc.dram_tensor("w_v", [L, INNER], F32, kind="ExternalInput").ap()
    w_out = nc.dram_tensor("w_out", [INNER, L], F32, kind="ExternalInput").ap()
    outT = nc.dram_tensor("outT", [L, QS], F32, kind="ExternalOutput").ap()

    # rearranged DRAM views (partition-major for SBUF loads)
    tok_T_v = tok_T.rearrange("(k p) t -> p k t", p=128)        # [128, 32, 1024]
    lq_v = lq_T.rearrange("(k p) q -> p k q", p=128)            # [128, 4, 512]
    w_q_v = w_q.rearrange("(k p) i -> p k i", p=128)            # [128, 4, 512]
    w_k_v = w_k.rearrange("(k p) i -> p k i", p=128)            # [128, 32, 512]
    w_v_v = w_v.rearrange("(k p) i -> p k i", p=128)            # [128, 32, 512]
    w_out_v = w_out.rearrange("(k p) l -> p k l", p=128)        # [128, 4, 4096]

    ld_eng = (lambda: nc.sync) if _USE_F32 else (lambda: nc.gpsimd)

    with tile.TileContext(nc) as tc:
        with (
            tc.tile_pool(name="persist", bufs=1) as persist,
            tc.tile_pool(name="dram", bufs=1, space="DRAM") as dram,
        ):
            # ---- persistent SBUF across phases ----
            qT_sb = persist.tile([64, H, QS], BF16, tag="qT")        # q^T per head
            kT_sb = persist.tile([128, 4, TS], BF16, tag="kT")       # k^T shard
            v_sb = persist.tile([128, TS // 128, INNER], BF16, tag="v")  # v shard
            aT_sb = persist.tile([128, 4, QS], FR, tag="aT")       # attn out^T
            idn = persist.tile([128, 128], BF16, tag="idn")          # identity
            ones_64 = persist.tile([1, D], F32, tag="ones64")

            # collective bounce buffers
            gk_in = dram.tile([INNER, TS], BF16, tag="gk_in")
            gk_out = dram.tile([N_CORES * INNER, TS], BF16, tag="gk_out")
            gv_in = dram.tile([TS, INNER], BF16, tag="gv_in")
            gv_out = dram.tile([N_CORES * TS, INNER], BF16, tag="gv_out")

            # identity matrix for PE transposes: idn[p, f] = (f == p)
            with tc.tile_pool(name="idpool", bufs=1) as idp:
                irow = idp.tile([128, 128], F32, tag="irow")
                icol = idp.tile([128, 1], F32, tag="icol")
                nc.gpsimd.iota(irow[:], pattern=[[1, 128]], base=0, channel_multiplier=0, allow_small_or_imprecise_dtypes=True)
                nc.gpsimd.iota(icol[:], pattern=[[0, 1]], base=0, channel_multiplier=1, allow_small_or_imprecise_dtypes=True)
                nc.vector.tensor_scalar(idn[:], irow[:], icol[:], None, EQ)
            nc.vector.memset(ones_64[:], 1.0)

            # ================= phase 1: projections =================
            if _PHASES in ("all", "proj"):
              with (
                  tc.tile_pool(name="proj", bufs=2) as proj,
                  tc.tile_pool(name="projq", bufs=1) as projq,
                  tc.tile_pool(name="pps", bufs=2, space="PSUM") as pps,
                  tc.tile_pool(name="ppq", bufs=1, space="PSUM") as ppq,
              ):
                  # --- q^T = w_q^T-contraction: lhsT=w_q tile, rhs=lq^T tile
                  wq_sb = projq.tile([128, 4, INNER], FR, tag="wq")
                  lq_sb = projq.tile([128, 4, QS], FR, tag="lq")
                  ld_eng().dma_start(wq_sb[:], w_q_v)
                  ld_eng().dma_start(lq_sb[:], lq_v)
                  for m in range(4):
                      ps = ppq.tile([128, QS], F32, tag="psq")
                      for kk in range(4):
                          nc.tensor.matmul(
                              ps[:],
                              wq_sb[:, kk, m * 128:(m + 1) * 128],
                              lq_sb[:, kk, :],
                              start=(kk == 0),
                              stop=(kk == 3),
                          )
                      qstage = projq.tile([128, QS], BF16, tag="qstage")
                      nc.vector.tensor_copy(qstage[:], ps[:])
                      # shift each head's 64 rows down to base partition 0
                      nc.sync.dma_start(qT_sb[:, 2 * m, :], qstage[0:64, :])
                      nc.sync.dma_start(qT_sb[:, 2 * m + 1, :], qstage[64:128, :])

                  # --- k^T and v^T projections, t streamed in two halves
                  for th in range(2):
                      tok_sb = proj.tile([128, 32, 512], FR, tag="tok", bufs=1)
                      ld_eng().dma_start(tok_sb[:], tok_T_v[:, :, th * 512:(th + 1) * 512])
                      # k^T [i-block m, t-half]
                      for m in range(4):
                          wcol = proj.tile([128, 32, 128], FR, tag="wcol")
                          ld_eng().dma_start(wcol[:], w_k_v[:, :, m * 128:(m + 1) * 128])
                          ps = pps.tile([128, 512], F32, tag="pp")
                          for k in range(32):
                              nc.tensor.matmul(
                                  ps[:], wcol[:, k, :], tok_sb[:, k, :],
                                  start=(k == 0), stop=(k == 31),
                              )
                          nc.vector.tensor_copy(kT_sb[:, m, th * 512:(th + 1) * 512], ps[:])
                      # v^T then transpose to v [t, i]
                      for m in range(4):
                          wcol = proj.tile([128, 32, 128], FR, tag="wcol")
                          ld_eng().dma_start(wcol[:], w_v_v[:, :, m * 128:(m + 1) * 128])
                          ps = pps.tile([128, 512], F32, tag="pp")
                          for k in range(32):
                              nc.tensor.matmul(
                                  ps[:], wcol[:, k, :], tok_sb[:, k, :],
                                  start=(k == 0), stop=(k == 31),
                              )
                          vst = proj.tile([128, 512], BF16, tag="vstage")
                          nc.vector.tensor_copy(vst[:], ps[:])
                          pt = pps.tile([128, 512], BF16, tag="pt")
                          for j in range(4):
                              nc.tensor.transpose(
                                  pt[:, j * 128:(j + 1) * 128],
                                  vst[:, j * 128:(j + 1) * 128],
                                  idn[:],
                              )
                          # pt columns j hold v[t-chunk j of this half, i-block m]
                          nc.vector.tensor_copy(
                              v_sb[:, th * 4:(th + 1) * 4, m * 128:(m + 1) * 128],
                              pt[:].rearrange("p (j i) -> p j i", j=4),
                          )

                  # bounce + gather
                  nc.sync.dma_start(
                      gk_in.rearrange("(m p) t -> p m t", p=128), kT_sb[:]
                  )
                  nc.sync.dma_start(
                      gv_in.rearrange("(j p) i -> p j i", p=128), v_sb[:]
                  )
                  if os.environ.get("BASSK_NO_CC"):
                      # timing-only variant: skip the collectives (wrong data)
                      nc.sync.dma_start(gk_out[0:INNER, :], gk_in[:])
                      nc.sync.dma_start(gv_out[0:TS, :], gv_in[:])
                  else:
                      nc.gpsimd.collective_compute(
                          "AllGather", mybir.AluOpType.bypass,
                          replica_groups=[list(range(N_CORES))],
                          ins=[gk_in.opt()], outs=[gk_out.opt()],
                      )
                      nc.gpsimd.collective_compute(
                          "AllGather", mybir.AluOpType.bypass,
                          replica_groups=[list(range(N_CORES))],
                          ins=[gv_in.opt()], outs=[gv_out.opt()],
                      )

            # ================= phase 2: attention =================
            if _PHASES == "attn":
                nc.vector.memset(qT_sb[:], 0.001)
            gv_v = gv_out.rearrange("(c j p) i -> c p j i", p=128, j=TS // 128)
            groups = [list(range(s, min(s + GRP, NT))) for s in range(0, NT, GRP)]

            if _PHASES in ("all", "attn"):
              with (
                tc.tile_pool(name="attn", bufs=2) as attn,
                tc.tile_pool(name="attn3", bufs=3) as attn3,
                  tc.tile_pool(name="aps", bufs=2, space="PSUM") as aps,
                  tc.tile_pool(name="aps1", bufs=1, space="PSUM") as aps1,
              ):
                  for h in range(H):
                      kTh = attn.tile([64, N_CORES, TS], BF16, tag="kTh")
                      for c in range(N_CORES):
                          nc.sync.dma_start(
                              kTh[:, c, :],
                              gk_out[c * INNER + h * D: c * INNER + h * D + D, :],
                          )
                      vh = attn.tile([128, NT, D + 1], BF16, tag="vh")
                      nc.vector.memset(vh[:, :, D], 1.0)
                      for c in range(N_CORES):
                          nc.sync.dma_start(
                              vh[:, c * (TS // 128):(c + 1) * (TS // 128), 0:D],
                              gv_v[c, :, :, h * D:(h + 1) * D],
                          )
                      qTh = qT_sb[:, h, :]

                      ps_o = aps1.tile([D + 1, QS], F32, tag="ps_o")
                      prev = None  # (group, pT tile)
                      for g in groups:
                          ps_s = aps.tile([128, GRP * QS], F32, tag="ps_s")
                          for jj, j in enumerate(g):
                              nc.tensor.matmul(
                                  ps_s[:, jj * QS:(jj + 1) * QS],
                                  kTh[:, j // (TS // 128), (j % (TS // 128)) * 128:
                                      (j % (TS // 128)) * 128 + 128],
                                  qTh,
                                  start=True, stop=True,
                              )
                          pT = attn3.tile([128, GRP * QS], BF16, tag="pT")
                          n = len(g) * QS
                          nc.scalar.activation(pT[:, 0:n], ps_s[:, 0:n], EXP, scale=SCALE)
                          if prev is not None:
                              pg, ppT = prev
                              for jj, j in enumerate(pg):
                                  nc.tensor.matmul(
                                      ps_o[:], vh[:, j, :], ppT[:, jj * QS:(jj + 1) * QS],
                                      start=(j == 0), stop=(j == NT - 1),
                                      skip_group_check=True,
                                  )
                          prev = (g, pT)
                      pg, ppT = prev
                      for jj, j in enumerate(pg):
                          nc.tensor.matmul(
                              ps_o[:], vh[:, j, :], ppT[:, jj * QS:(jj + 1) * QS],
                              start=(j == 0), stop=(j == NT - 1),
                              skip_group_check=True,
                          )

                      # normalize: a^T = u^T / denom  (denom broadcast via PE)
                      u_sb = attn.tile([D + 1, QS], F32, tag="u")
                      nc.vector.tensor_copy(u_sb[:], ps_o[:])
                      dn0 = attn.tile([1, QS], F32, tag="dn0")
                      nc.sync.dma_start(dn0[:], u_sb[D:D + 1, :])  # shift to partition 0
                      recip = attn.tile([1, QS], F32, tag="recip")
                      nc.vector.reciprocal(recip[:], dn0[:])
                      ps_r = aps.tile([D, QS], F32, tag="ps_s")  # borrow a ps_s slot
                      nc.tensor.matmul(ps_r[:], ones_64[:], recip[:], start=True, stop=True)
                      a_tmp = attn.tile([D, QS], F32, tag="a_tmp")
                      nc.vector.tensor_mul(a_tmp[:], u_sb[0:D, :], ps_r[:])
                      ld_eng().dma_start(
                          aT_sb[(h % 2) * 64:(h % 2) * 64 + 64, h // 2, :], a_tmp[:]
                      )

            # ================= phase 3: output projection =================
            if _PHASES in ("all", "attn"):
              with (
                tc.tile_pool(name="outp", bufs=3) as outp,
                tc.tile_pool(name="ops", bufs=2, space="PSUM") as ops,
              ):
                  for m in range(L // 128):
                      wo = outp.tile([128, 4, 128], FR, tag="wo")
                      ld_eng().dma_start(wo[:], w_out_v[:, :, m * 128:(m + 1) * 128])
                      ps = ops.tile([128, QS], F32, tag="po")
                      for kk in range(4):
                          nc.tensor.matmul(
                              ps[:], wo[:, kk, :], aT_sb[:, kk, :],
                              start=(kk == 0), stop=(kk == 3),
                          )
                      of = outp.tile([128, QS], F32, tag="of")
                      nc.vector.tensor_copy(of[:], ps[:])
                      nc.sync.dma_start(outT[m * 128:(m + 1) * 128, :], of[:])

    nc.compile()
    return nc


_COMPILED = None


def _get_compiled():
    global _COMPILED
    if _COMPILED is None:
        _COMPILED = build_program()
    return _COMPILED


def make_in_maps(token_input, learned_queries, w_q, w_k, w_v, w_out):
    token_input = np.ascontiguousarray(np.asarray(token_input, dtype=np.float32))
    learned_queries = np.ascontiguousarray(np.asarray(learned_queries, dtype=np.float32))
    w_q = np.ascontiguousarray(np.asarray(w_q, dtype=np.float32))
    w_k = np.ascontiguousarray(np.asarray(w_k, dtype=np.float32))
    w_v = np.ascontiguousarray(np.asarray(w_v, dtype=np.float32))
    w_out = np.ascontiguousarray(np.asarray(w_out, dtype=np.float32))
    in_maps = []
    for c in range(N_CORES):
        in_maps.append({
            "tok_T": np.ascontiguousarray(token_input[c * TS:(c + 1) * TS, :].T),
            "lq_T": np.ascontiguousarray(learned_queries[c * QS:(c + 1) * QS, :].T),
            "w_q": w_q, "w_k": w_k, "w_v": w_v, "w_out": w_out,
        })
    return in_maps


def assemble(results):
    out = np.empty((V, L), dtype=np.float32)
    for c in range(N_CORES):
        out[c * QS:(c + 1) * QS, :] = results[c]["outT"].T
    return out


def kernel(token_input, learned_queries, w_q, w_k, w_v, w_out):
    nc = _get_compiled()
    in_maps = make_in_maps(token_input, learned_queries, w_q, w_k, w_v, w_out)
    res = run_bass_kernel_spmd(nc, in_maps, list(range(N_CORES)))
    return assemble(res.results)


# revision 9
# speedup vs baseline: 67.8930x; 2.5328x over previous
"""Cross-attention reducer kernel for Trainium2, 8 NeuronCores (SPMD).

Problem (full shapes):
    token_input    [T=8192, L=4096]
    learned_queries[V=4096, I=512]
    w_q [I, I], w_k [L, I], w_v [L, I], w_out [I, L]

    q = learned_queries @ w_q;  k = token_input @ w_k;  v = token_input @ w_v
    per head h (H=8, D=64): attn = softmax(q_h k_h^T / sqrt(D)); out_h = attn @ v_h
    out = concat_h(out_h) @ w_out      -> [V, L]

Sharding: queries (V) are sharded 8 ways; the K/V projections are
sequence-parallel (each core projects its T/8 token shard with the full
w_k/w_v) followed by an AllGather of k^T and v, after which every core runs
attention for all 8 heads over its own 512 queries and its full-T gathered
k/v, then applies the output projection for its V-shard. The final output is
computed transposed (final^T = w_out^T-contraction) so every matmul contracts
on the partition dimension with no large transposes anywhere:

    q^T  [I, Vs]  = w_q (lhsT)  x lq^T (rhs)
    k^T  [I, t]   = w_k (lhsT)  x tok^T (rhs)        (per T-shard, gathered)
    v^T  [I, t]   = w_v (lhsT)  x tok^T (rhs), then 128x128 PE-transposes
                    to v [t, I] before the gather
    s^T  [t, Vs]  = k_h^T (lhsT) x q_h^T (rhs)       (t-tiles of 128)
    p^T           = exp(s^T / 8)                      (no max-subtraction:
                    scores are O(3), exp can't overflow; identical math)
    u^T  [D+1,Vs] = [v_h | 1] (lhsT) x p^T (rhs)     (row D = softmax denom)
    a^T  [D, Vs]  = u^T * (1/denom broadcast via PE outer product)
    out^T[L, Vs]  = w_out (lhsT) x a^T (rhs)

Matmul dtypes: fp32r (full-rate fp32, ~6.5e-5 rel err) for all projections
and the output projection; bf16 for the gathered k/v + attention matmuls
(halves gather traffic; ~0.5% worst-case contribution, well under tolerance).
"""

import os

import numpy as np

import concourse.bacc as bacc
import concourse.tile as tile
import concourse.mybir as mybir
from concourse.bass_utils import run_bass_kernel_spmd

F32 = mybir.dt.float32
F32R = mybir.dt.float32r
BF16 = mybir.dt.bfloat16
EXP = mybir.ActivationFunctionType.Exp
EQ = mybir.AluOpType.is_equal

N_CORES = 8
T, L, V, INNER = 8192, 4096, 4096, 512
H, D = 8, 64
TS = T // N_CORES      # 1024  t-shard per core
QS = V // N_CORES      # 512   query shard per core
SCALE = D ** -0.5      # 0.125

NT = T // 128          # 64 t-tiles per head
GRP = 3                # t-tiles per exp batch (3 psum banks)

# diagnostics: BASSK_F32=1 -> plain f32 matmuls + sync-engine loads (no casts)
_USE_F32 = bool(os.environ.get("BASSK_F32"))
_PHASES = os.environ.get("BASSK_PHASES", "all")  # all | proj | attn


def build_program():
    FR = F32 if _USE_F32 else F32R
    ld = None  # set after nc exists
    nc = bacc.Bacc(
        "TRN2", target_bir_lowering=False, debug=False, num_devices=N_CORES
    )

    tok_T = nc.dram_tensor("tok_T", [L, TS], F32, kind="ExternalInput").ap()
    lq_T = nc.dram_tensor("lq_T", [INNER, QS], F32, kind="ExternalInput").ap()
    w_q = nc.dram_tensor("w_q", [INNER, INNER], F32, kind="ExternalInput").ap()
    w_k = nc.dram_tensor("w_k", [L, INNER], F32, kind="ExternalInput").ap()
    w_v = nc.dram_tensor("w_v", [L, INNER], F32, kind="ExternalInput").ap()
    w_out = nc.dram_tensor("w_out", [INNER, L], F32, kind="ExternalInput").ap()
    outT = nc.dram_tensor("outT", [L, QS], F32, kind="ExternalOutput").ap()

    # rearranged DRAM views (partition-major for SBUF loads)
    tok_T_v = tok_T.rearrange("(k p) t -> p k t", p=128)        # [128, 32, 1024]
    lq_v = lq_T.rearrange("(k p) q -> p k q", p=128)            # [128, 4, 512]
    w_q_v = w_q.rearrange("(k p) i -> p k i", p=128)            # [128, 4, 512]
    w_k_v = w_k.rearrange("(k p) i -> p k i", p=128)            # [128, 32, 512]
    w_v_v = w_v.rearrange("(k p) i -> p k i", p=128)            # [128, 32, 512]
    w_out_v = w_out.rearrange("(k p) l -> p k l", p=128)        # [128, 4, 4096]

    ld_eng = (lambda: nc.sync) if _USE_F32 else (lambda: nc.gpsimd)

    with tile.TileContext(nc) as tc:
        with (
            tc.tile_pool(name="persist", bufs=1) as persist,
            tc.tile_pool(name="dram", bufs=1, space="DRAM") as dram,
        ):
            # ---- persistent SBUF across phases ----
            qT_sb = persist.tile([64, H, QS], BF16, tag="qT")        # q^T per head
            kT_sb = persist.tile([128, 4, TS], BF16, tag="kT")       # k^T shard
            v_sb = persist.tile([128, TS // 128, INNER], BF16, tag="v")  # v shard
            aT_sb = persist.tile([128, 4, QS], FR, tag="aT")       # attn out^T
            idn = persist.tile([128, 128], BF16, tag="idn")          # identity
            ones_64 = persist.tile([1, D], F32, tag="ones64")

            # collective bounce buffers
            gk_in = dram.tile([INNER, TS], BF16, tag="gk_in")
            gk_out = dram.tile([N_CORES * INNER, TS], BF16, tag="gk_out")
            gv_in = dram.tile([TS, INNER], BF16, tag="gv_in")
            gv_out = dram.tile([N_CORES * TS, INNER], BF16, tag="gv_out")

            # identity matrix for PE transposes: idn[p, f] = (f == p)
            with tc.tile_pool(name="idpool", bufs=1) as idp:
                irow = idp.tile([128, 128], F32, tag="irow")
                icol = idp.tile([128, 1], F32, tag="icol")
                nc.gpsimd.iota(irow[:], pattern=[[1, 128]], base=0, channel_multiplier=0, allow_small_or_imprecise_dtypes=True)
                nc.gpsimd.iota(icol[:], pattern=[[0, 1]], base=0, channel_multiplier=1, allow_small_or_imprecise_dtypes=True)
                nc.vector.tensor_scalar(idn[:], irow[:], icol[:], None, EQ)
            nc.vector.memset(ones_64[:], 1.0)

            # ================= phase 1: projections =================
            if _PHASES in ("all", "proj"):
              with (
                  tc.tile_pool(name="proj", bufs=2) as proj,
                  tc.tile_pool(name="projq", bufs=1) as projq,
                  tc.tile_pool(name="pps", bufs=2, space="PSUM") as pps,
                  tc.tile_pool(name="ppq", bufs=1, space="PSUM") as ppq,
              ):
                  # --- q^T = w_q^T-contraction: lhsT=w_q tile, rhs=lq^T tile
                  wq_sb = projq.tile([128, 4, INNER], FR, tag="wq")
                  lq_sb = projq.tile([128, 4, QS], FR, tag="lq")
                  ld_eng().dma_start(wq_sb[:], w_q_v)
                  ld_eng().dma_start(lq_sb[:], lq_v)
                  for m in range(4):
                      ps = ppq.tile([128, QS], F32, tag="psq")
                      for kk in range(4):
                          nc.tensor.matmul(
                              ps[:],
                              wq_sb[:, kk, m * 128:(m + 1) * 128],
                              lq_sb[:, kk, :],
                              start=(kk == 0),
                              stop=(kk == 3),
                          )
                      qstage = projq.tile([128, QS], BF16, tag="qstage")
                      nc.vector.tensor_copy(qstage[:], ps[:])
                      # shift each head's 64 rows down to base partition 0
                      nc.sync.dma_start(qT_sb[:, 2 * m, :], qstage[0:64, :])
                      nc.sync.dma_start(qT_sb[:, 2 * m + 1, :], qstage[64:128, :])

                  # --- k^T and v^T projections, t streamed in two halves
                  for th in range(2):
                      tok_sb = proj.tile([128, 32, 512], FR, tag="tok", bufs=1)
                      ld_eng().dma_start(tok_sb[:], tok_T_v[:, :, th * 512:(th + 1) * 512])
                      # k^T [i-block m, t-half]
                      for m in range(4):
                          wcol = proj.tile([128, 32, 128], FR, tag="wcol")
                          ld_eng().dma_start(wcol[:], w_k_v[:, :, m * 128:(m + 1) * 128])
                          ps = pps.tile([128, 512], F32, tag="pp")
                          for k in range(32):
                              nc.tensor.matmul(
                                  ps[:], wcol[:, k, :], tok_sb[:, k, :],
                                  start=(k == 0), stop=(k == 31),
                              )
                          nc.vector.tensor_copy(kT_sb[:, m, th * 512:(th + 1) * 512], ps[:])
                      # v^T then transpose to v [t, i]
                      for m in range(4):
                          wcol = proj.tile([128, 32, 128], FR, tag="wcol")
                          ld_eng().dma_start(wcol[:], w_v_v[:, :, m * 128:(m + 1) * 128])
                          ps = pps.tile([128, 512], F32, tag="pp")
                          for k in range(32):
                              nc.tensor.matmul(
                                  ps[:], wcol[:, k, :], tok_sb[:, k, :],
                                  start=(k == 0), stop=(k == 31),
                              )
                          vst = proj.tile([128, 512], BF16, tag="vstage")
                          nc.vector.tensor_copy(vst[:], ps[:])
                          pt = pps.tile([128, 512], BF16, tag="pt")
                          for j in range(4):
                              nc.tensor.transpose(
                                  pt[:, j * 128:(j + 1) * 128],
                                  vst[:, j * 128:(j + 1) * 128],
                                  idn[:],
                              )
                          # pt columns j hold v[t-chunk j of this half, i-block m]
                          nc.vector.tensor_copy(
                              v_sb[:, th * 4:(th + 1) * 4, m * 128:(m + 1) * 128],
                              pt[:].rearrange("p (j i) -> p j i", j=4),
                          )

                  # bounce + gather
                  nc.sync.dma_start(
                      gk_in.rearrange("(m p) t -> p m t", p=128), kT_sb[:]
                  )
                  nc.sync.dma_start(
                      gv_in.rearrange("(j p) i -> p j i", p=128), v_sb[:]
                  )
                  if os.environ.get("BASSK_NO_CC"):
                      # timing-only variant: skip the collectives (wrong data)
                      nc.sync.dma_start(gk_out[0:INNER, :], gk_in[:])
                      nc.sync.dma_start(gv_out[0:TS, :], gv_in[:])
                  else:
                      nc.gpsimd.collective_compute(
                          "AllGather", mybir.AluOpType.bypass,
                          replica_groups=[list(range(N_CORES))],
                          ins=[gk_in.opt()], outs=[gk_out.opt()],
                      )
                      nc.gpsimd.collective_compute(
                          "AllGather", mybir.AluOpType.bypass,
                          replica_groups=[list(range(N_CORES))],
                          ins=[gv_in.opt()], outs=[gv_out.opt()],
                      )

            # ================= phase 2: attention =================
            if _PHASES == "attn":
                nc.vector.memset(qT_sb[:], 0.001)
            gv_v = gv_out.rearrange("(c j p) i -> c p j i", p=128, j=TS // 128)
            groups = [list(range(s, min(s + GRP, NT))) for s in range(0, NT, GRP)]

            if _PHASES in ("all", "attn"):
              with (
                tc.tile_pool(name="attn", bufs=2) as attn,
                tc.tile_pool(name="attn3", bufs=3) as attn3,
                  tc.tile_pool(name="aps", bufs=2, space="PSUM") as aps,
                  tc.tile_pool(name="aps1", bufs=1, space="PSUM") as aps1,
              ):
                  # all heads' V loaded once with contiguous 1KB runs
                  vh_all = attn.tile([128, NT, INNER], BF16, tag="vh_all", bufs=1)
                  for c in range(N_CORES):
                      nc.sync.dma_start(
                          vh_all[:, c * (TS // 128):(c + 1) * (TS // 128), :],
                          gv_v[c, :, :, :],
                      )
                  for h in range(H):
                      kTh = attn.tile([64, N_CORES, TS], BF16, tag="kTh")
                      for c in range(N_CORES):
                          nc.sync.dma_start(
                              kTh[:, c, :],
                              gk_out[c * INNER + h * D: c * INNER + h * D + D, :],
                          )
                      vh = attn.tile([128, NT, D + 1], BF16, tag="vh")
                      nc.vector.memset(vh[:, :, D], 1.0)
                      nc.vector.tensor_copy(
                          vh[:, :, 0:D], vh_all[:, :, h * D:(h + 1) * D]
                      )
                      qTh = qT_sb[:, h, :]

                      ps_o = aps1.tile([D + 1, QS], F32, tag="ps_o")
                      prev = None  # (group, pT tile)
                      for g in groups:
                          ps_s = aps.tile([128, GRP * QS], F32, tag="ps_s")
                          for jj, j in enumerate(g):
                              nc.tensor.matmul(
                                  ps_s[:, jj * QS:(jj + 1) * QS],
                                  kTh[:, j // (TS // 128), (j % (TS // 128)) * 128:
                                      (j % (TS // 128)) * 128 + 128],
                                  qTh,
                                  start=True, stop=True,
                              )
                          pT = attn3.tile([128, GRP * QS], BF16, tag="pT")
                          n = len(g) * QS
                          nc.scalar.activation(pT[:, 0:n], ps_s[:, 0:n], EXP, scale=SCALE)
                          if prev is not None:
                              pg, ppT = prev
                              for jj, j in enumerate(pg):
                                  nc.tensor.matmul(
                                      ps_o[:], vh[:, j, :], ppT[:, jj * QS:(jj + 1) * QS],
                                      start=(j == 0), stop=(j == NT - 1),
                                      skip_group_check=True,
                                  )
                          prev = (g, pT)
                      pg, ppT = prev
                      for jj, j in enumerate(pg):
                          nc.tensor.matmul(
                              ps_o[:], vh[:, j, :], ppT[:, jj * QS:(jj + 1) * QS],
                              start=(j == 0), stop=(j == NT - 1),
                              skip_group_check=True,
                          )

                      # normalize: a^T = u^T / denom  (denom broadcast via PE)
                      u_sb = attn.tile([D + 1, QS], F32, tag="u")
                      nc.vector.tensor_copy(u_sb[:], ps_o[:])
                      dn0 = attn.tile([1, QS], F32, tag="dn0")
                      nc.sync.dma_start(dn0[:], u_sb[D:D + 1, :])  # shift to partition 0
                      recip = attn.tile([1, QS], F32, tag="recip")
                      nc.vector.reciprocal(recip[:], dn0[:])
                      ps_r = aps.tile([D, QS], F32, tag="ps_s")  # borrow a ps_s slot
                      nc.tensor.matmul(ps_r[:], ones_64[:], recip[:], start=True, stop=True)
                      a_tmp = attn.tile([D, QS], F32, tag="a_tmp")
                      nc.vector.tensor_mul(a_tmp[:], u_sb[0:D, :], ps_r[:])
                      ld_eng().dma_start(
                          aT_sb[(h % 2) * 64:(h % 2) * 64 + 64, h // 2, :], a_tmp[:]
                      )

            # ================= phase 3: output projection =================
            if _PHASES in ("all", "attn"):
              with (
                tc.tile_pool(name="outp", bufs=3) as outp,
                tc.tile_pool(name="ops", bufs=2, space="PSUM") as ops,
              ):
                  for m in range(L // 128):
                      wo = outp.tile([128, 4, 128], FR, tag="wo")
                      ld_eng().dma_start(wo[:], w_out_v[:, :, m * 128:(m + 1) * 128])
                      ps = ops.tile([128, QS], F32, tag="po")
                      for kk in range(4):
                          nc.tensor.matmul(
                              ps[:], wo[:, kk, :], aT_sb[:, kk, :],
                              start=(kk == 0), stop=(kk == 3),
                          )
                      of = outp.tile([128, QS], F32, tag="of")
                      nc.vector.tensor_copy(of[:], ps[:])
                      nc.sync.dma_start(outT[m * 128:(m + 1) * 128, :], of[:])

    nc.compile()
    return nc


_COMPILED = None


def _get_compiled():
    global _COMPILED
    if _COMPILED is None:
        _COMPILED = build_program()
    return _COMPILED


def make_in_maps(token_input, learned_queries, w_q, w_k, w_v, w_out):
    token_input = np.ascontiguousarray(np.asarray(token_input, dtype=np.float32))
    learned_queries = np.ascontiguousarray(np.asarray(learned_queries, dtype=np.float32))
    w_q = np.ascontiguousarray(np.asarray(w_q, dtype=np.float32))
    w_k = np.ascontiguousarray(np.asarray(w_k, dtype=np.float32))
    w_v = np.ascontiguousarray(np.asarray(w_v, dtype=np.float32))
    w_out = np.ascontiguousarray(np.asarray(w_out, dtype=np.float32))
    in_maps = []
    for c in range(N_CORES):
        in_maps.append({
            "tok_T": np.ascontiguousarray(token_input[c * TS:(c + 1) * TS, :].T),
            "lq_T": np.ascontiguousarray(learned_queries[c * QS:(c + 1) * QS, :].T),
            "w_q": w_q, "w_k": w_k, "w_v": w_v, "w_out": w_out,
        })
    return in_maps


def assemble(results):
    out = np.empty((V, L), dtype=np.float32)
    for c in range(N_CORES):
        out[c * QS:(c + 1) * QS, :] = results[c]["outT"].T
    return out


def kernel(token_input, learned_queries, w_q, w_k, w_v, w_out):
    nc = _get_compiled()
    in_maps = make_in_maps(token_input, learned_queries, w_q, w_k, w_v, w_out)
    res = run_bass_kernel_spmd(nc, in_maps, list(range(N_CORES)))
    return assemble(res.results)
